# revision 27
# baseline (speedup 1.0000x reference)
"""Multi-head attention (16 heads, d_model=1024, B=2, S=2048) on 8 Trainium2
NeuronCores, tensor-parallel over heads (2 heads per core).

Per-core program (matmuls bf16, fp32 PSUM). Key empirical HW rules this
design is built around (measured via microbenchmarks, not the cost model):
  - a matmul whose stationary operand differs from the previous one pays a
    serial ~150-280ns Ldweights penalty; consecutive matmuls SHARING the
    stationary are nearly free -> every stationary is used for 2 multiplies
    (two 512-token query chunks / two PSUM banks).
  - PSUM accumulation state is per-bank: start=True zeroes the whole bank,
    so every accumulation chain owns a full bank.
Structure:
  - q_T/k_T/v_T = (W X^T + b) in transposed [dpc, token] layout, f-outer
    with the weight tile stationary across two 512-token chunks.
  - v is turned token-major ([tok, dk] + ones column per head) with XBAR
    DMA transposes, giving the V-stationary attention matmul its stationary
    operand and free softmax denominators.
  - attention runs h-serialized half-passes (head h, query-chunk pair):
    scores_T[j, q] = k-stationary QK (one k load, 2 chunk multiplies),
    exp on ScalarE straight out of PSUM (scores ~ N(0,1), no max needed),
    AV with v_aug stationary accumulating [65, 512] per chunk over the 16
    key tiles (row 64 = softmax denominator).
  - normalize via DVE reciprocal + gpsimd partition-broadcast + multiply
    into attn_c[dpc, tok] (no ScalarE Ln/Exp chain -> no activation-table
    reloads); Wo row-block then gives a partial [B*S, 1024] per core.
Host: sum of the 8 partials + (bv @ Wo^T + bo) correction (exact because
softmax rows sum to 1, so the V-bias commutes out of attention).
"""

import os
import contextlib

import numpy as np
import ml_dtypes

import concourse.bass as bass
import concourse.tile as tile
import concourse.bacc as bacc
from concourse import mybir
from concourse import bass_utils

BF16 = ml_dtypes.bfloat16

D_MODEL = 1024
NUM_HEADS = 16
DK = 64
B, S = 2, 2048
BS = B * S
N_CORES = 8
HPC = NUM_HEADS // N_CORES          # heads per core = 2
DPC = HPC * DK                      # head-dim slice per core = 128
P = 128
NF = D_MODEL // P                   # 8 contraction tiles for projections
SJT = S // P                        # 16 key tiles per batch
QW = 512                            # token chunk width
NQC = S // QW                       # 4 chunks per batch
NVT = S // P                        # 16 v token tiles per batch

f32 = mybir.dt.float32
bf16 = mybir.dt.bfloat16

OUTF32 = os.environ.get("OUTF32", "0") == "1"
out_dt = f32 if OUTF32 else bf16
OUT_NP = np.float32 if OUTF32 else BF16

LABELS = {}


def _lbl(ins, *label):
    try:
        LABELS[ins.ins.name] = label
    except Exception:
        pass


def _emit(tc, aps, loop=1):
    nc = tc.nc
    xq, xk, xv, wq, wk, wv, wo, bq, bk, out = aps
    SK = int(os.environ.get("SK", "4"))
    XBUFS = int(os.environ.get("XBUFS", "13"))
    ETBUFS = int(os.environ.get("ETBUFS", "8"))
    WOEV = os.environ.get("WOEV", "vector")

    with contextlib.ExitStack() as ctx:
        const = ctx.enter_context(tc.tile_pool(name="const", bufs=1))
        persist = ctx.enter_context(tc.tile_pool(name="persist", bufs=1))
        xpool = ctx.enter_context(tc.tile_pool(name="xpool", bufs=XBUFS))
        et_pool = ctx.enter_context(tc.tile_pool(name="et", bufs=ETBUFS))
        attn_pool = ctx.enter_context(tc.tile_pool(name="attn", bufs=8))
        rc_pool = ctx.enter_context(tc.tile_pool(name="rc", bufs=4))
        bc_pool = ctx.enter_context(tc.tile_pool(name="bc", bufs=4))
        vt_pool = ctx.enter_context(tc.tile_pool(name="vt", bufs=3))
        ot_pool = ctx.enter_context(tc.tile_pool(name="ot", bufs=2))
        pp_pair = ctx.enter_context(
            tc.tile_pool(name="pp_pair", bufs=int(os.environ.get("PPPAIR", "2")),
                         space="PSUM"))
        pp_av = ctx.enter_context(
            tc.tile_pool(name="pp_av", bufs=int(os.environ.get("PPAV", "2")),
                         space="PSUM"))
        pp_blk = ctx.enter_context(
            tc.tile_pool(name="pp_blk", bufs=int(os.environ.get("PPBLK", "2")),
                         space="PSUM"))

        # ---- constants ----
        wq_sb = const.tile([P, NF, P], bf16)
        wk_sb = const.tile([P, NF, P], bf16)
        wv_sb = const.tile([P, NF, P], bf16)
        wo_sb = const.tile([P, D_MODEL], bf16)
        bq_sb = const.tile([P, 1], f32)
        bk_sb = const.tile([P, 1], f32)

        def _load_w(which):
            for w_sb, w_ap in ((wq_sb, wq), (wk_sb, wk), (wv_sb, wv)):
                if w_sb is which:
                    nc.sync.dma_start(
                        w_sb[:], w_ap.rearrange("p (n m) -> p n m", n=NF))

        _load_w(wq_sb)
        nc.sync.dma_start(bq_sb[:], bq[:])

        q_sb = persist.tile([P, BS], bf16)
        k_sb = persist.tile([P, BS], bf16)
        v_sb = persist.tile([P, B * NVT, HPC * (DK + 1)], bf16)
        # ones columns of v_aug (softmax denominator rows)
        nc.vector.memset(v_sb[:, :, DK : DK + 1], 1.0)
        nc.vector.memset(v_sb[:, :, 2 * DK + 1 : 2 * DK + 2], 1.0)

        loop_cm = tc.For_i(0, loop, 1) if loop > 1 else contextlib.nullcontext()
        with loop_cm:
            xt = {}

            def load_x(b):
                # one DMA per (stream, 512-token chunk): [128, 8, 512] tile
                if b == 0:
                    order = [("q", 0), ("q", 1), ("k", 0), ("k", 1),
                             ("v", 0), ("v", 1), ("k", 2), ("k", 3),
                             ("v", 2), ("v", 3), ("q", 2), ("q", 3)]
                else:
                    order = [("k", 0), ("k", 1), ("q", 0), ("q", 1),
                             ("v", 0), ("v", 1), ("k", 2), ("k", 3),
                             ("v", 2), ("v", 3), ("q", 2), ("q", 3)]
                src = {"q": xq, "k": xk, "v": xv}
                for i_ord, (s, c) in enumerate(order):
                    if b == 0 and i_ord == 2:
                        _load_w(wk_sb)
                        nc.sync.dma_start(bk_sb[:], bk[:])
                    if b == 0 and i_ord == 4:
                        _load_w(wv_sb)
                        nc.sync.dma_start(wo_sb[:], wo[:])
                    a = src[s][:, b * S + c * QW : b * S + (c + 1) * QW]
                    t = xpool.tile([P, NF, QW], bf16, tag="x", name=f"x{s}{b}c{c}")
                    nc.sync.dma_start(t[:], a.rearrange("(n p) m -> p n m", p=P))
                    xt[(s, b, c)] = t

            # ---- deferred-work queue with paced draining ----
            pending = []
            done = set()
            spent = [0]

            def run(key, fn, cost):
                fn()
                done.add(key)
                spent[0] += cost

            def push(key, fn, cost):
                pending.append((key, fn, cost))

            def force(key):
                if key in done:
                    return
                for i, (k, fn, cost) in enumerate(pending):
                    if k == key:
                        pending.pop(i)
                        run(k, fn, cost)
                        return
                raise KeyError(key)

            def drain_paced(target, allow_wo):
                while pending and spent[0] < target:
                    hit = None
                    for i, (k, fn, cost) in enumerate(pending):
                        if not allow_wo and k[0] == "wo":
                            continue
                        hit = i
                        break
                    if hit is None:
                        return
                    k, fn, cost = pending.pop(hit)
                    run(k, fn, cost)

            def drain_all():
                while pending:
                    k, fn, cost = pending.pop(0)
                    run(k, fn, cost)

            # f-outer projection: weight tile stationary across 2 chunks
            def emit_qk_proj(kind, b, cp):
                w_sb, b_sb, dest = ((wq_sb, bq_sb, q_sb) if kind == "q"
                                    else (wk_sb, bk_sb, k_sb))
                ps = [pp_blk.tile([P, QW], f32, tag="blk", name=f"ps{j}")
                      for j in range(2)]
                for f in range(NF):
                    for j in range(2):
                        _lbl(nc.tensor.matmul(
                            ps[j][:], w_sb[:, f, :],
                            xt[(kind, b, 2 * cp + j)][:, f, :],
                            start=(f == 0), stop=(f == NF - 1)),
                            kind + "proj", b, cp, f, j)
                for j in range(2):
                    c = 2 * cp + j
                    nc.vector.tensor_scalar_add(
                        dest[:, b * S + c * QW : b * S + (c + 1) * QW],
                        ps[j][:], b_sb[:])

            def emit_v_proj(b, cp):
                # dpc-major projection, then XBAR-transpose into token-major
                # v_aug tiles (strided around the ones columns)
                ps = [pp_blk.tile([P, QW], f32, tag="blk", name=f"ps{j}")
                      for j in range(2)]
                for f in range(NF):
                    for j in range(2):
                        _lbl(nc.tensor.matmul(
                            ps[j][:], wv_sb[:, f, :],
                            xt[("v", b, 2 * cp + j)][:, f, :],
                            start=(f == 0), stop=(f == NF - 1)),
                            "vproj", b, cp, f, j)
                vt = vt_pool.tile([P, 2, QW], bf16, tag="vt")
                for j in range(2):
                    nc.vector.tensor_copy(vt[:, j, :], ps[j][:])
                for j in range(2):
                    for i2 in range(QW // P):
                        t = (2 * cp + j) * 4 + i2
                        dst = v_sb[:, b * NVT + t, 0:DK]
                        dst = bass.AP(dst.tensor, dst.offset,
                                      [dst.ap[0], [DK + 1, 2], [1, DK]])
                        nc.sync.dma_start_transpose(
                            dst, vt[:, j, i2 * P : (i2 + 1) * P])

            def emit_wo(bi, qc, attn_c):
                ot = ot_pool.tile([P, NQC, D_MODEL], out_dt, tag="ot")
                for i2 in range(QW // P):
                    po = [pp_blk.tile([P, QW], f32, tag="blk", name=f"po{j}")
                          for j in range(2)]
                    st = attn_c[:, i2 * P : (i2 + 1) * P]
                    for half in range(2):
                        _lbl(nc.tensor.matmul(
                            po[half][:], st,
                            wo_sb[:, half * QW : (half + 1) * QW],
                            start=True, stop=True), "wo", bi, qc, i2, half)
                    for half in range(2):
                        dsl = ot[:, i2, half * QW : (half + 1) * QW]
                        if WOEV == "scalar":
                            nc.scalar.copy(dsl, po[half][:])
                        else:
                            nc.vector.tensor_copy(dsl, po[half][:])
                row0 = bi * S + qc * QW
                nc.sync.dma_start(
                    out[row0 : row0 + QW, :].rearrange("(a p) m -> p a m", p=P),
                    ot[:])

            # ---- schedule ----
            load_x(0)
            for b in range(B):
                for cp in range(2):
                    push(("q", b, cp),
                         (lambda b=b, cp=cp: emit_qk_proj("q", b, cp)), 4000)
                    push(("k", b, cp),
                         (lambda b=b, cp=cp: emit_qk_proj("k", b, cp)), 4000)
                    push(("v", b, cp),
                         (lambda b=b, cp=cp: emit_v_proj(b, cp)), 4400)

            filler_total = B * (3 * 2 * 4400 + NQC * 2200)
            nsteps_total = B * HPC * 2 * (SJT + SK)
            step_no = [0]
            attn_cs = {}

            for bi in range(B):
                for h in range(HPC):
                    for hp in range(2):
                        if (bi, h, hp) == (0, 0, 1):
                            load_x(1)
                        force(("q", bi, hp))
                        c0, c1 = 2 * hp, 2 * hp + 1
                        if h == 0 and hp == 0:
                            for qc in range(NQC):
                                attn_cs[(bi, qc)] = attn_pool.tile(
                                    [P, QW], bf16, tag="attn",
                                    name=f"attn{bi}_{qc}")
                        hsl = slice(h * DK, (h + 1) * DK)
                        avs = [pp_av.tile([DK + 1, QW], f32, tag="av",
                                          name=f"av{j}") for j in range(2)]
                        ets = {}
                        for jt in range(SJT + SK):
                            if jt < SJT:
                                force(("k", bi, 0))
                                if jt >= 6:
                                    force(("k", bi, 1))
                                if h == 0 and hp == 0:
                                    for la in range(2 * SK + 2):
                                        if jt + la < NVT:
                                            force(("v", bi, (jt + la) // 8))
                                jsl = slice(bi * S + jt * P,
                                            bi * S + (jt + 1) * P)
                                pair = pp_pair.tile([P, 2, QW], f32, tag="pair")
                                kst = k_sb[hsl, jsl]
                                for j, c in ((0, c0), (1, c1)):
                                    _lbl(nc.tensor.matmul(
                                        pair[:, j, :], kst,
                                        q_sb[hsl, bi * S + c * QW :
                                             bi * S + (c + 1) * QW],
                                        start=True, stop=True),
                                        "qk", bi, h, hp, jt, j)
                                et = et_pool.tile([P, 2, QW], bf16, tag="et")
                                nc.scalar.activation(
                                    et[:], pair[:],
                                    mybir.ActivationFunctionType.Exp,
                                    scale=0.125)
                                ets[jt] = et
                            ja = jt - SK
                            if ja >= 0:
                                et = ets.pop(ja)
                                vst = v_sb[:, bi * NVT + ja,
                                           h * (DK + 1) : (h + 1) * (DK + 1)]
                                for j in range(2):
                                    _lbl(nc.tensor.matmul(
                                        avs[j][:], vst, et[:, j, :],
                                        start=(ja == 0), stop=(ja == SJT - 1)),
                                        "av", bi, h, hp, ja, j)
                            step_no[0] += 1
                            drain_paced(
                                filler_total * step_no[0] // nsteps_total,
                                allow_wo=(jt >= 2))
                        # half-pass tail: normalize into attn_c rows
                        for j, c in ((0, c0), (1, c1)):
                            rc = rc_pool.tile([1, QW], f32, tag="rc")
                            nc.vector.reciprocal(rc[:], avs[j][DK : DK + 1, :])
                            bc = bc_pool.tile([DK, QW], f32, tag="bc")
                            nc.gpsimd.partition_broadcast(bc[:], rc[:])
                            nc.vector.tensor_mul(
                                attn_cs[(bi, c)][hsl, :], avs[j][0:DK, :],
                                bc[:])
                        if h == HPC - 1:
                            last = (bi == B - 1) and (hp == 1)
                            for j, c in ((0, c0), (1, c1)):
                                push(("wo", bi, c),
                                     (lambda bi=bi, c=c,
                                      a=attn_cs[(bi, c)]: emit_wo(bi, c, a)),
                                     2200)
                                if last:
                                    force(("wo", bi, c))

            drain_all()


def _build(loop=1):
    nc = bacc.Bacc("TRN2", target_bir_lowering=False, debug=False,
                   num_devices=N_CORES)
    xq = nc.dram_tensor("xq_t", [D_MODEL, BS], bf16, kind="ExternalInput").ap()
    xk = nc.dram_tensor("xk_t", [D_MODEL, BS], bf16, kind="ExternalInput").ap()
    xv = nc.dram_tensor("xv_t", [D_MODEL, BS], bf16, kind="ExternalInput").ap()
    wq = nc.dram_tensor("wq_t", [P, D_MODEL], bf16, kind="ExternalInput").ap()
    wk = nc.dram_tensor("wk_t", [P, D_MODEL], bf16, kind="ExternalInput").ap()
    wv = nc.dram_tensor("wv_t", [P, D_MODEL], bf16, kind="ExternalInput").ap()
    wo = nc.dram_tensor("wo_t", [DPC, D_MODEL], bf16, kind="ExternalInput").ap()
    bq = nc.dram_tensor("bq", [DPC, 1], f32, kind="ExternalInput").ap()
    bk = nc.dram_tensor("bk", [DPC, 1], f32, kind="ExternalInput").ap()
    out = nc.dram_tensor("out_p", [BS, D_MODEL], out_dt,
                         kind="ExternalOutput").ap()

    with tile.TileContext(nc) as tc:
        _emit(tc, (xq, xk, xv, wq, wk, wv, wo, bq, bk, out), loop=loop)
    nc.compile()
    return nc


_cache = {}


def _get_nc(loop=1):
    key = (loop,) + tuple(
        os.environ.get(k, "") for k in
        ("SK", "XBUFS", "ETBUFS", "WOEV", "PPPAIR", "PPAV", "PPBLK", "OUTF32"))
    if key not in _cache:
        _cache[key] = _build(loop)
    return _cache[key]


def _wprep(w_slice):
    # [dpc, D] weight slice -> [128, 8, 128] = (ctr%128, ctr//128, dpc) laid
    # out contiguously per partition so the load uses 2KB descriptors.
    wt = np.ascontiguousarray(np.asarray(w_slice).T)   # [D, dpc]
    wt = wt.reshape(NF, P, DPC).transpose(1, 0, 2)     # [128, 8, 128]
    return np.ascontiguousarray(wt.reshape(P, D_MODEL)).astype(BF16)


def _make_in_maps(Q, K, V, Wq, bq, Wk, bk, Wv, bv, Wo, bo):
    xq_t = np.ascontiguousarray(
        np.asarray(Q, np.float32).reshape(BS, D_MODEL).T).astype(BF16)
    xk_t = np.ascontiguousarray(
        np.asarray(K, np.float32).reshape(BS, D_MODEL).T).astype(BF16)
    xv_t = np.ascontiguousarray(
        np.asarray(V, np.float32).reshape(BS, D_MODEL).T).astype(BF16)
    in_maps = []
    for c in range(N_CORES):
        sl = slice(c * DPC, (c + 1) * DPC)
        in_maps.append({
            "xq_t": xq_t, "xk_t": xk_t, "xv_t": xv_t,
            "wq_t": _wprep(np.asarray(Wq)[sl]),
            "wk_t": _wprep(np.asarray(Wk)[sl]),
            "wv_t": _wprep(np.asarray(Wv)[sl]),
            "wo_t": np.ascontiguousarray(np.asarray(Wo)[:, sl].T).astype(BF16),
            "bq": np.asarray(bq, np.float32)[sl].reshape(DPC, 1).copy(),
            "bk": np.asarray(bk, np.float32)[sl].reshape(DPC, 1).copy(),
        })
    return in_maps


def kernel(Q, K, V, Wq, bq, Wk, bk, Wv, bv, Wo, bo):
    nc = _get_nc()
    in_maps = _make_in_maps(Q, K, V, Wq, bq, Wk, bk, Wv, bv, Wo, bo)
    res = bass_utils.run_bass_kernel_spmd(nc, in_maps, core_ids=list(range(N_CORES)))
    acc = np.zeros((BS, D_MODEL), np.float32)
    for c in range(N_CORES):
        acc += np.asarray(res.results[c]["out_p"], np.float32)
    corr = (np.asarray(bv, np.float64) @ np.asarray(Wo, np.float64).T
            + np.asarray(bo, np.float64)).astype(np.float32)
    return (acc + corr[None, :]).reshape(B, S, D_MODEL).astype(np.float32)


# revision 28
# speedup vs baseline: 1.2113x; 1.2113x over previous
"""Multi-head attention (16 heads, d_model=1024, B=2, S=2048) on 8 Trainium2
NeuronCores, tensor-parallel over heads (2 heads per core).

Per-core program (all matmuls bf16 with fp32 PSUM accumulation):
  - q_T/k_T = (W X^T + b) computed in transposed [d, token] layout
  - v in natural [token, d] layout with a ones-column appended (gives the
    softmax denominators for free from the same attn@v matmul)
  - scores_T[j, q] = k_T^T-stationary matmul, exp on ScalarE straight out of
    PSUM (softmax without max-subtraction: scores ~ N(0,1), no overflow risk)
  - unnormalized attn output + denominators accumulate in PSUM; normalization
    applied during eviction via a partition-broadcast reciprocal
  - row block of Wo produces a partial [B*S, 1024] output per core
Host: sum of the 8 partials + (bv @ Wo^T + bo) correction (exact because
softmax rows sum to 1, so the V-bias commutes out of attention).
"""

import numpy as np
import ml_dtypes

import concourse.bass as bass
import concourse.tile as tile
import concourse.bacc as bacc
from concourse import mybir
from concourse import bass_utils

BF16 = ml_dtypes.bfloat16

D_MODEL = 1024
NUM_HEADS = 16
DK = 64
B, S = 2, 2048
BS = B * S
N_CORES = 8
HPC = NUM_HEADS // N_CORES          # heads per core = 2
DPC = HPC * DK                      # head-dim slice per core = 128
P = 128
NF = D_MODEL // P                   # 8 contraction tiles for projections
NIT = BS // P                       # 32 token tiles of 128
SJT = S // P                        # 16 key tiles per batch
FREE = 1024                         # moving free-dim for bf16 matmuls
NQC = BS // FREE                    # 4 projection column chunks
NQT = S // FREE                     # 2 query chunks per batch

f32 = mybir.dt.float32
bf16 = mybir.dt.bfloat16


def _emit(tc, aps, loop=1):
    nc = tc.nc
    xq, xk, xv, wq, wk, wv, wo, bq, bk, out = aps
    QW = 512                       # attention query-chunk width
    NQC2 = S // QW                 # 4 chunks per batch

    import contextlib
    with contextlib.ExitStack() as ctx:
        const = ctx.enter_context(tc.tile_pool(name="const", bufs=1))
        xpool = ctx.enter_context(tc.tile_pool(name="xpool", bufs=17))
        persist = ctx.enter_context(tc.tile_pool(name="persist", bufs=1))
        exp_pool = ctx.enter_context(tc.tile_pool(name="exp", bufs=8))
        attn_pool = ctx.enter_context(tc.tile_pool(name="attnp", bufs=6))
        bc_pool = ctx.enter_context(tc.tile_pool(name="bcast", bufs=3))
        rc_pool = ctx.enter_context(tc.tile_pool(name="recip", bufs=3))
        un_pool = ctx.enter_context(tc.tile_pool(name="unnorm", bufs=4))
        out_pool = ctx.enter_context(tc.tile_pool(name="outp", bufs=2))
        pp_pair = ctx.enter_context(tc.tile_pool(name="pp_pair", bufs=int(__import__("os").environ.get("PPPAIR", "2")), space="PSUM"))
        pp_av = ctx.enter_context(tc.tile_pool(name="pp_av", bufs=int(__import__("os").environ.get("PPAV", "3")), space="PSUM"))
        pp_blk = ctx.enter_context(tc.tile_pool(name="pp_blk", bufs=int(__import__("os").environ.get("PPBLK", "1")), space="PSUM"))

        # ---- constants ----
        wq_sb = const.tile([P, NF, P], bf16)
        wk_sb = const.tile([P, NF, P], bf16)
        wv_sb = const.tile([P, NF, P], bf16)
        wo_sb = const.tile([P, D_MODEL], bf16)
        for w_sb, w_ap in ((wq_sb, wq), (wk_sb, wk), (wv_sb, wv)):
            nc.sync.dma_start(w_sb[:], w_ap.rearrange("(n p) m -> p n m", p=P))
        nc.sync.dma_start(wo_sb[:], wo[:])
        bq_sb = const.tile([P, 1], f32)
        bk_sb = const.tile([P, 1], f32)
        nc.sync.dma_start(bq_sb[:], bq[:])
        nc.sync.dma_start(bk_sb[:], bk[:])

        q_sb = persist.tile([P, BS], bf16)
        k_sb = persist.tile([P, BS], bf16)
        v_sb = persist.tile([P, NIT, 2 * (DK + 1)], bf16)

        import os as _osc
        cet = None
        if _osc.environ.get("CONSTET") == "1":
            cet = const.tile([P, HPC, 512], bf16)
            nc.vector.memset(cet[:], 0.5)

        # ones columns of v_aug (softmax denominator rows)
        nc.vector.memset(v_sb[:, :, DK : DK + 1], 1.0)
        nc.vector.memset(v_sb[:, :, 2 * DK + 1 : 2 * DK + 2], 1.0)

        import os as _osf
        NOX = _osf.environ.get("NOX") == "1"
        NOPROJ = _osf.environ.get("NOPROJ") == "1"
        NOWO = _osf.environ.get("NOWO") == "1"
        NOCHAIN = _osf.environ.get("NOCHAIN") == "1"
        if NOPROJ:
            nc.vector.memset(q_sb[:], 0.1)
            nc.vector.memset(k_sb[:], 0.1)
            nc.vector.memset(v_sb[:], 0.1)
        if NOCHAIN:
            pass

        import contextlib as _ctl
        loop_cm = tc.For_i(0, loop, 1) if loop > 1 else _ctl.nullcontext()
        with loop_cm:
            xt = {}

            def load_x(b):
                QWC = S // QW
                srcq = xq[:, b * S : (b + 1) * S].rearrange("(n p) m -> n p m", p=P)
                for name, x_ap in (("k", xk), ("v", xv)):
                    src = x_ap[:, b * S : (b + 1) * S].rearrange("(n p) m -> n p m", p=P)
                    tiles = []
                    for f in range(NF):
                        t = xpool.tile([P, S], bf16, tag="x")
                        nc.sync.dma_start(t[:], src[f])
                        tiles.append(t)
                    xt[(name, b)] = tiles
                    if name == "k":
                        # first q chunk right after k, before the bulk of v
                        qtiles = [[None] * QWC for _ in range(NF)]
                        xt[("q", b)] = qtiles
                        for f in range(NF):
                            t = xpool.tile([P, QW], bf16, tag="xq", bufs=34, name=f"xq{f}c0")
                            nc.sync.dma_start(t[:], srcq[f, :, 0:QW])
                            qtiles[f][0] = t
                for c in range(1, QWC):
                    for f in range(NF):
                        t = xpool.tile([P, QW], bf16, tag="xq", bufs=34, name=f"xq{f}c{c}")
                        nc.sync.dma_start(t[:], srcq[f, :, c * QW : (c + 1) * QW])
                        xt[("q", b)][f][c] = t

            def emit_qk(kind, b, c):
                w_sb, b_sb, dest = ((wq_sb, bq_sb, q_sb) if kind == "q"
                                    else (wk_sb, bk_sb, k_sb))
                ps = pp_blk.tile([P, QW], f32, tag="blk")
                cs = slice(c * QW, (c + 1) * QW)  # local within batch
                for f in range(NF):
                    rhs = (xt[(kind, b)][f][c][:]
                           if kind == "q" else xt[(kind, b)][f][:, cs])
                    nc.tensor.matmul(ps[:], w_sb[:, f, :], rhs,
                                     start=(f == 0), stop=(f == NF - 1))
                nc.vector.tensor_scalar_add(
                    dest[:, b * S + c * QW : b * S + (c + 1) * QW], ps[:], b_sb[:])

            def emit_v(b, it2):
                ps = pp_blk.tile([P, QW], f32, tag="blk")
                isl = slice(it2 * P, (it2 + 1) * P)
                for f in range(NF):
                    nc.tensor.matmul(ps[:, 0:P], xt[("v", b)][f][:, isl], wv_sb[:, f, :],
                                     start=(f == 0), stop=(f == NF - 1))
                dst = v_sb[:, b * SJT + it2, 0:DK]
                dst = bass.AP(dst.tensor, dst.offset, [dst.ap[0], [DK + 1, 2], [1, DK]])
                nc.vector.tensor_copy(dst, ps[:, 0:P].rearrange("p (a b) -> p a b", a=2))

            def emit_wo(attn_c, b, qc, i2):
                po = pp_blk.tile([P, QW], f32, tag="blk")
                nc.tensor.matmul(po[:], attn_c[:, i2 * P : (i2 + 1) * P],
                                 wo_sb[:, 0:QW], start=True, stop=True)
                po2 = pp_blk.tile([P, QW], f32, tag="blk")
                nc.tensor.matmul(po2[:], attn_c[:, i2 * P : (i2 + 1) * P],
                                 wo_sb[:, QW:], start=True, stop=True)
                ot = out_pool.tile([P, D_MODEL], f32)
                import os as _osw
                if _osw.environ.get("WOACT", "0") == "1":
                    nc.scalar.copy(ot[:, 0:QW], po[:])
                    nc.scalar.copy(ot[:, QW:], po2[:])
                else:
                    nc.vector.tensor_copy(ot[:, 0:QW], po[:])
                    nc.vector.tensor_copy(ot[:, QW:], po2[:])
                row0 = b * S + qc * QW + i2 * P
                nc.sync.dma_start(out[row0 : row0 + P, :], ot[:])

            pending = []
            done = set()

            def emit_block(blk):
                key = blk[:1] + tuple(x for x in blk[1:] if not hasattr(x, "tensor"))
                if blk[0] == "qk_q":
                    emit_qk("q", blk[1], blk[2])
                elif blk[0] == "qk_k":
                    emit_qk("k", blk[1], blk[2])
                elif blk[0] == "v":
                    emit_v(blk[1], blk[2])
                else:
                    emit_wo(blk[1], blk[2], blk[3], blk[4])
                done.add(key)

            def force(key):
                if NOPROJ or key in done:
                    return
                for i, blk in enumerate(pending):
                    bkey = blk[:1] + tuple(x for x in blk[1:] if not hasattr(x, "tensor"))
                    if bkey == key:
                        pending.pop(i)
                        emit_block(blk)
                        return
                raise KeyError(key)

            def drain(n):
                for _ in range(min(n, len(pending))):
                    emit_block(pending.pop(0))

            for b in range(B):
                if not NOX:
                    load_x(b)
                if not NOPROJ:
                    for c in range(NQC2):
                        pending.append(("qk_k", b, c))
                    pending.append(("qk_q", b, 0))
                    for it2 in range(SJT):
                        pending.append(("v", b, it2))
                    for c in range(1, NQC2):
                        pending.append(("qk_q", b, c))

            import os as _os2
            if _os2.environ.get("PROJONLY") == "1":
                drain(len(pending))
                return
            for b in range(B):
                # prologue for this batch: k fully, first q chunk
                for c in range(NQC2):
                    force(("qk_k", b, c))

                for qc in range(NQC2):
                    force(("qk_q", b, qc))
                    qss = slice(b * S + qc * QW, b * S + (qc + 1) * QW)
                    attn_c = attn_pool.tile([P, QW], bf16, tag="attn")
                    pav = [pp_av.tile([DK + 1, QW], f32, tag="av", name=f"pav{h}")
                           for h in range(HPC)]
                    SKEW = int(__import__("os").environ.get("SKEW", "2"))
                    ets = {}
                    for jt in range(SJT + SKEW):
                        if jt < SJT:
                            for la in range(SKEW + 1):
                                if jt + la < SJT:
                                    force(("v", b, jt + la))
                            jsl = slice(b * S + jt * P, b * S + (jt + 1) * P)
                            pair = pp_pair.tile([P, HPC, QW], f32, tag="pair")
                            for h in range(HPC):
                                nc.tensor.matmul(
                                    pair[:, h, :], k_sb[h * DK : (h + 1) * DK, jsl],
                                    q_sb[h * DK : (h + 1) * DK, qss],
                                    start=True, stop=True,
                                )
                            import os as _os3
                            if _os3.environ.get("CONSTET") == "1":
                                ets[jt] = cet
                            else:
                                et = exp_pool.tile([P, HPC, QW], bf16)
                                if _os3.environ.get("EXPSPLIT") == "1":
                                    for h in range(HPC):
                                        nc.scalar.activation(
                                            et[:, h, :], pair[:, h, :],
                                            mybir.ActivationFunctionType.Exp, scale=0.125,
                                        )
                                else:
                                    nc.scalar.activation(
                                        et[:], pair[:],
                                        mybir.ActivationFunctionType.Exp, scale=0.125,
                                    )
                                ets[jt] = et
                        ja = jt - SKEW
                        if ja >= 0:
                            et = ets.pop(ja)
                            for h in range(HPC):
                                nc.tensor.matmul(
                                    pav[h][:],
                                    v_sb[:, b * SJT + ja, h * (DK + 1) : (h + 1) * (DK + 1)],
                                    et[:, h, :],
                                    start=(ja == 0), stop=(ja == SJT - 1),
                                )
                        drain(1)
                        if jt == 9:
                            nb, nqc = (b, qc + 1) if qc + 1 < NQC2 else (b + 1, 0)
                            if nb < B:
                                force(("qk_q", nb, nqc))
                    for h in range(HPC):
                        if NOCHAIN:
                            continue
                        import os as _os
                        if _os.environ.get("LNCHAIN", "0") == "1":
                            # reciprocal of the softmax sums via exp(-ln(s)) on
                            # ScalarE (same activation-table set as the score
                            # exp), keeping the slow DVE InstReciprocal off the
                            # critical path entirely.
                            lnr = rc_pool.tile([1, QW], f32)
                            nc.scalar.activation(
                                lnr[:], pav[h][DK : DK + 1, :],
                                mybir.ActivationFunctionType.Ln)
                            rec = rc_pool.tile([1, QW], f32, name="rec")
                            nc.scalar.activation(
                                rec[:], lnr[:],
                                mybir.ActivationFunctionType.Exp, scale=-1.0)
                            bc = bc_pool.tile([DK, QW], f32)
                            nc.gpsimd.partition_broadcast(bc[:], rec[:])
                            nc.vector.tensor_mul(
                                attn_c[h * DK : (h + 1) * DK, :], pav[h][0:DK, :], bc[:])
                            continue
                        if _os.environ.get("DIVCHAIN", "0") == "1":
                            # evict pav to SBUF first: frees the PSUM slot after
                            # one op; the normalization chain then runs on SBUF
                            # with no PSUM slot held.
                            un = un_pool.tile([DK + 1, QW], f32)
                            nc.scalar.copy(un[:], pav[h][:])
                            rc = rc_pool.tile([1, QW], f32)
                            nc.vector.reciprocal(rc[:], un[DK : DK + 1, :])
                            bc = bc_pool.tile([DK, QW], f32)
                            nc.gpsimd.partition_broadcast(bc[:], rc[:])
                            nc.vector.tensor_mul(
                                attn_c[h * DK : (h + 1) * DK, :], un[0:DK, :], bc[:])
                            continue
                        rc = rc_pool.tile([1, QW], f32)
                        nc.vector.reciprocal(rc[:], pav[h][DK : DK + 1, :])
                        bc = bc_pool.tile([DK, QW], f32)
                        if _os.environ.get("NOBCAST") == "1":
                            nc.vector.memset(bc[:], 1.0)   # timing-only variant
                        else:
                            nc.gpsimd.partition_broadcast(bc[:], rc[:])
                        nc.vector.tensor_mul(
                            attn_c[h * DK : (h + 1) * DK, :], pav[h][0:DK, :], bc[:])
                    if not NOWO:
                        for i2 in range(QW // P):
                            pending.append(("wo", attn_c, b, qc, i2))

            drain(len(pending))


def _build(loop=1):
    nc = bacc.Bacc("TRN2", target_bir_lowering=False, debug=False,
                   num_devices=N_CORES)
    xq = nc.dram_tensor("xq_t", [D_MODEL, BS], bf16, kind="ExternalInput").ap()
    xk = nc.dram_tensor("xk_t", [D_MODEL, BS], bf16, kind="ExternalInput").ap()
    xv = nc.dram_tensor("xv_t", [D_MODEL, BS], bf16, kind="ExternalInput").ap()
    wq = nc.dram_tensor("wq_t", [D_MODEL, DPC], bf16, kind="ExternalInput").ap()
    wk = nc.dram_tensor("wk_t", [D_MODEL, DPC], bf16, kind="ExternalInput").ap()
    wv = nc.dram_tensor("wv_t", [D_MODEL, DPC], bf16, kind="ExternalInput").ap()
    wo = nc.dram_tensor("wo_t", [DPC, D_MODEL], bf16, kind="ExternalInput").ap()
    bq = nc.dram_tensor("bq", [DPC, 1], f32, kind="ExternalInput").ap()
    bk = nc.dram_tensor("bk", [DPC, 1], f32, kind="ExternalInput").ap()
    out = nc.dram_tensor("out_p", [BS, D_MODEL], f32, kind="ExternalOutput").ap()

    with tile.TileContext(nc) as tc:
        _emit(tc, (xq, xk, xv, wq, wk, wv, wo, bq, bk, out), loop=loop)
    nc.compile()
    return nc


_cache = {}


def _get_nc(loop=1):
    import os
    key = (loop, os.environ.get("SKEW", "2"), os.environ.get("PPPAIR", "2"),
           os.environ.get("PPAV", "3"), os.environ.get("PPBLK", "1"), os.environ.get("NOEXP", ""), os.environ.get("EXPSPLIT", ""), os.environ.get("CONSTET", ""), os.environ.get("PROJONLY", ""), os.environ.get("NOX", ""), os.environ.get("NOPROJ", ""), os.environ.get("NOWO", ""), os.environ.get("NOCHAIN", ""), os.environ.get("DIVCHAIN", "0"), os.environ.get("LNCHAIN", "0"), os.environ.get("WOACT", "0"), os.environ.get("LNCHAIN", "0"))
    if key not in _cache:
        _cache[key] = _build(loop)
    return _cache[key]


def _make_in_maps(Q, K, V, Wq, bq, Wk, bk, Wv, bv, Wo, bo):
    xq_t = np.ascontiguousarray(np.asarray(Q, np.float32).reshape(BS, D_MODEL).T).astype(BF16)
    xk_t = np.ascontiguousarray(np.asarray(K, np.float32).reshape(BS, D_MODEL).T).astype(BF16)
    xv_t = np.ascontiguousarray(np.asarray(V, np.float32).reshape(BS, D_MODEL).T).astype(BF16)
    in_maps = []
    for c in range(N_CORES):
        sl = slice(c * DPC, (c + 1) * DPC)
        in_maps.append({
            "xq_t": xq_t, "xk_t": xk_t, "xv_t": xv_t,
            "wq_t": np.ascontiguousarray(np.asarray(Wq)[sl].T).astype(BF16),
            "wk_t": np.ascontiguousarray(np.asarray(Wk)[sl].T).astype(BF16),
            "wv_t": np.ascontiguousarray(np.asarray(Wv)[sl].T).astype(BF16),
            "wo_t": np.ascontiguousarray(np.asarray(Wo)[:, sl].T).astype(BF16),
            "bq": np.asarray(bq, np.float32)[sl].reshape(DPC, 1).copy(),
            "bk": np.asarray(bk, np.float32)[sl].reshape(DPC, 1).copy(),
        })
    return in_maps


def kernel(Q, K, V, Wq, bq, Wk, bk, Wv, bv, Wo, bo):
    nc = _get_nc()
    in_maps = _make_in_maps(Q, K, V, Wq, bq, Wk, bk, Wv, bv, Wo, bo)
    res = bass_utils.run_bass_kernel_spmd(nc, in_maps, core_ids=list(range(N_CORES)))
    acc = np.zeros((BS, D_MODEL), np.float32)
    for c in range(N_CORES):
        acc += res.results[c]["out_p"]
    corr = (np.asarray(bv, np.float64) @ np.asarray(Wo, np.float64).T
            + np.asarray(bo, np.float64)).astype(np.float32)
    return (acc + corr[None, :]).reshape(B, S, D_MODEL).astype(np.float32)



# revision 35
# speedup vs baseline: 1.2567x; 1.0374x over previous
"""Multi-head attention (16 heads, d_model=1024, B=2, S=2048) on 8 Trainium2
NeuronCores, tensor-parallel over heads (2 heads per core).

Per-core program (all matmuls bf16 with fp32 PSUM accumulation):
  - q_T/k_T = (W X^T + b) computed in transposed [d, token] layout
  - v in natural [token, d] layout with a ones-column appended (gives the
    softmax denominators for free from the same attn@v matmul)
  - scores_T[j, q] = k_T^T-stationary matmul, exp on ScalarE straight out of
    PSUM (softmax without max-subtraction: scores ~ N(0,1), no overflow risk)
  - unnormalized attn output + denominators accumulate in PSUM; normalization
    applied during eviction via a partition-broadcast reciprocal
  - row block of Wo produces a partial [B*S, 1024] output per core
Host: sum of the 8 partials + (bv @ Wo^T + bo) correction (exact because
softmax rows sum to 1, so the V-bias commutes out of attention).
"""

import numpy as np
import ml_dtypes

import concourse.bass as bass
import concourse.tile as tile
import concourse.bacc as bacc
from concourse import mybir
from concourse import bass_utils

BF16 = ml_dtypes.bfloat16

D_MODEL = 1024
NUM_HEADS = 16
DK = 64
B, S = 2, 2048
BS = B * S
N_CORES = 8
HPC = NUM_HEADS // N_CORES          # heads per core = 2
DPC = HPC * DK                      # head-dim slice per core = 128
P = 128
NF = D_MODEL // P                   # 8 contraction tiles for projections
NIT = BS // P                       # 32 token tiles of 128
SJT = S // P                        # 16 key tiles per batch
FREE = 1024                         # moving free-dim for bf16 matmuls
NQC = BS // FREE                    # 4 projection column chunks
NQT = S // FREE                     # 2 query chunks per batch

f32 = mybir.dt.float32
bf16 = mybir.dt.bfloat16


def _emit(tc, aps, loop=1):
    nc = tc.nc
    xq, xk, xv, wq, wk, wv, wo, bq, bk, out = aps
    QW = 512                       # attention query-chunk width
    NQC2 = S // QW                 # 4 chunks per batch

    import contextlib
    with contextlib.ExitStack() as ctx:
        const = ctx.enter_context(tc.tile_pool(name="const", bufs=1))
        xpool = ctx.enter_context(tc.tile_pool(name="xpool", bufs=15))
        persist = ctx.enter_context(tc.tile_pool(name="persist", bufs=1))
        exp_pool = ctx.enter_context(tc.tile_pool(name="exp", bufs=8))
        attn_pool = ctx.enter_context(tc.tile_pool(name="attnp", bufs=6))
        bc_pool = ctx.enter_context(tc.tile_pool(name="bcast", bufs=3))
        rc_pool = ctx.enter_context(tc.tile_pool(name="recip", bufs=3))
        un_pool = ctx.enter_context(tc.tile_pool(name="unnorm", bufs=4))
        out_pool = ctx.enter_context(tc.tile_pool(name="outp", bufs=2))
        pp_pair = ctx.enter_context(tc.tile_pool(name="pp_pair", bufs=int(__import__("os").environ.get("PPPAIR", "2")), space="PSUM"))
        pp_av = ctx.enter_context(tc.tile_pool(name="pp_av", bufs=int(__import__("os").environ.get("PPAV", "3")), space="PSUM"))
        pp_blk = ctx.enter_context(tc.tile_pool(name="pp_blk", bufs=int(__import__("os").environ.get("PPBLK", "1")), space="PSUM"))

        # ---- constants ----
        wq_sb = const.tile([P, NF, P], bf16)
        wk_sb = const.tile([P, NF, P], bf16)
        wv_sb = const.tile([P, NF, P], bf16)
        wo_sb = const.tile([P, D_MODEL], bf16)
        for w_sb, w_ap in ((wq_sb, wq), (wk_sb, wk), (wv_sb, wv)):
            nc.sync.dma_start(w_sb[:], w_ap.rearrange("p (n m) -> p n m", n=NF))
        nc.sync.dma_start(wo_sb[:], wo[:])
        bq_sb = const.tile([P, 1], f32)
        bk_sb = const.tile([P, 1], f32)
        nc.sync.dma_start(bq_sb[:], bq[:])
        nc.sync.dma_start(bk_sb[:], bk[:])

        q0_sb = persist.tile([P, BS], bf16)
        q1_sb = persist.tile([P, BS], bf16)
        nc.vector.memset(q0_sb[DK:P, :], 0.0)
        nc.vector.memset(q1_sb[0:DK, :], 0.0)
        k_sb = persist.tile([P, BS], bf16)
        v_sb = persist.tile([P, NIT, 2 * (DK + 1)], bf16)

        import os as _osc
        cet = None
        if _osc.environ.get("CONSTET") == "1":
            cet = const.tile([P, HPC, 512], bf16)
            nc.vector.memset(cet[:], 0.5)

        # ones columns of v_aug (softmax denominator rows)
        nc.vector.memset(v_sb[:, :, DK : DK + 1], 1.0)
        nc.vector.memset(v_sb[:, :, 2 * DK + 1 : 2 * DK + 2], 1.0)

        import os as _osf
        NOX = _osf.environ.get("NOX") == "1"
        NOPROJ = _osf.environ.get("NOPROJ") == "1"
        NOWO = _osf.environ.get("NOWO") == "1"
        NOCHAIN = _osf.environ.get("NOCHAIN") == "1"
        if NOPROJ:
            nc.vector.memset(q0_sb[0:DK, :], 0.1)
            nc.vector.memset(q1_sb[DK:P, :], 0.1)
            nc.vector.memset(k_sb[:], 0.1)
            nc.vector.memset(v_sb[:], 0.1)
        if NOCHAIN:
            pass

        import contextlib as _ctl
        loop_cm = tc.For_i(0, loop, 1) if loop > 1 else _ctl.nullcontext()
        with loop_cm:
            xt = {}

            def load_x(b):
                QWC = S // QW
                srcq = xq[:, b * S : (b + 1) * S].rearrange("(n p) m -> n p m", p=P)
                for name, x_ap in (("k", xk), ("v", xv)):
                    src = x_ap[:, b * S : (b + 1) * S].rearrange("(n p) m -> n p m", p=P)
                    tiles = []
                    for f in range(NF):
                        t = xpool.tile([P, S], bf16, tag="x")
                        nc.sync.dma_start(t[:], src[f])
                        tiles.append(t)
                    xt[(name, b)] = tiles
                    if name == "k":
                        # first q chunk right after k, before the bulk of v
                        qtiles = [[None] * QWC for _ in range(NF)]
                        xt[("q", b)] = qtiles
                        for f in range(NF):
                            t = xpool.tile([P, QW], bf16, tag="xq", bufs=34, name=f"xq{f}c0")
                            nc.sync.dma_start(t[:], srcq[f, :, 0:QW])
                            qtiles[f][0] = t
                for c in range(1, QWC):
                    for f in range(NF):
                        t = xpool.tile([P, QW], bf16, tag="xq", bufs=34, name=f"xq{f}c{c}")
                        nc.sync.dma_start(t[:], srcq[f, :, c * QW : (c + 1) * QW])
                        xt[("q", b)][f][c] = t

            def emit_qk(kind, b, c):
                w_sb, b_sb = ((wq_sb, bq_sb) if kind == "q"
                              else (wk_sb, bk_sb))
                ps = pp_blk.tile([P, QW], f32, tag="blk")
                cs = slice(c * QW, (c + 1) * QW)  # local within batch
                for f in range(NF):
                    rhs = (xt[(kind, b)][f][c][:]
                           if kind == "q" else xt[(kind, b)][f][:, cs])
                    nc.tensor.matmul(ps[:], w_sb[:, f, :], rhs,
                                     start=(f == 0), stop=(f == NF - 1))
                gs = slice(b * S + c * QW, b * S + (c + 1) * QW)
                if kind == "q":
                    nc.vector.tensor_scalar_add(
                        q0_sb[0:DK, gs], ps[0:DK, :], bq_sb[0:DK])
                    nc.vector.tensor_scalar_add(
                        q1_sb[DK:P, gs], ps[DK:P, :], bq_sb[DK:P])
                else:
                    nc.vector.tensor_scalar_add(k_sb[:, gs], ps[:], b_sb[:])

            def emit_v(b, it2):
                ps = pp_blk.tile([P, QW], f32, tag="blk")
                isl = slice(it2 * P, (it2 + 1) * P)
                for f in range(NF):
                    nc.tensor.matmul(ps[:, 0:P], xt[("v", b)][f][:, isl], wv_sb[:, f, :],
                                     start=(f == 0), stop=(f == NF - 1))
                dst = v_sb[:, b * SJT + it2, 0:DK]
                dst = bass.AP(dst.tensor, dst.offset, [dst.ap[0], [DK + 1, 2], [1, DK]])
                nc.vector.tensor_copy(dst, ps[:, 0:P].rearrange("p (a b) -> p a b", a=2))

            def emit_wo(attn_c, b, qc, i2):
                po = pp_blk.tile([P, QW], f32, tag="blk")
                nc.tensor.matmul(po[:], attn_c[:, i2 * P : (i2 + 1) * P],
                                 wo_sb[:, 0:QW], start=True, stop=True)
                po2 = pp_blk.tile([P, QW], f32, tag="blk")
                nc.tensor.matmul(po2[:], attn_c[:, i2 * P : (i2 + 1) * P],
                                 wo_sb[:, QW:], start=True, stop=True)
                ot = out_pool.tile([P, D_MODEL], f32)
                import os as _osw
                if _osw.environ.get("WOACT", "0") == "1":
                    nc.scalar.copy(ot[:, 0:QW], po[:])
                    nc.scalar.copy(ot[:, QW:], po2[:])
                else:
                    nc.vector.tensor_copy(ot[:, 0:QW], po[:])
                    nc.vector.tensor_copy(ot[:, QW:], po2[:])
                row0 = b * S + qc * QW + i2 * P
                nc.sync.dma_start(out[row0 : row0 + P, :], ot[:])

            pending = []
            done = set()

            def emit_block(blk):
                key = blk[:1] + tuple(x for x in blk[1:] if not hasattr(x, "tensor"))
                if blk[0] == "qk_q":
                    emit_qk("q", blk[1], blk[2])
                elif blk[0] == "qk_k":
                    emit_qk("k", blk[1], blk[2])
                elif blk[0] == "v":
                    emit_v(blk[1], blk[2])
                else:
                    emit_wo(blk[1], blk[2], blk[3], blk[4])
                done.add(key)

            def force(key):
                if NOPROJ or key in done:
                    return
                for i, blk in enumerate(pending):
                    bkey = blk[:1] + tuple(x for x in blk[1:] if not hasattr(x, "tensor"))
                    if bkey == key:
                        pending.pop(i)
                        emit_block(blk)
                        return
                raise KeyError(key)

            def drain(n):
                for _ in range(min(n, len(pending))):
                    emit_block(pending.pop(0))

            for b in range(B):
                if not NOX:
                    load_x(b)
                if not NOPROJ:
                    for c in range(NQC2):
                        pending.append(("qk_k", b, c))
                    pending.append(("qk_q", b, 0))
                    for it2 in range(SJT):
                        pending.append(("v", b, it2))
                    for c in range(1, NQC2):
                        pending.append(("qk_q", b, c))

            import os as _os2
            if _os2.environ.get("PROJONLY") == "1":
                drain(len(pending))
                return
            for b in range(B):
                # prologue for this batch: k fully, first q chunk
                for c in range(NQC2):
                    force(("qk_k", b, c))

                for qc in range(NQC2):
                    force(("qk_q", b, qc))
                    qss = slice(b * S + qc * QW, b * S + (qc + 1) * QW)
                    attn_c = attn_pool.tile([P, QW], bf16, tag="attn")
                    pav = [pp_av.tile([DK + 1, QW], f32, tag="av", name=f"pav{h}")
                           for h in range(HPC)]
                    SKEW = int(__import__("os").environ.get("SKEW", "2"))
                    ets = {}
                    for jt in range(SJT + SKEW):
                        if jt < SJT:
                            for la in range(SKEW + 1):
                                if jt + la < SJT:
                                    force(("v", b, jt + la))
                            jsl = slice(b * S + jt * P, b * S + (jt + 1) * P)
                            pair = pp_pair.tile([P, HPC, QW], f32, tag="pair")
                            for h, qp in ((0, q0_sb), (1, q1_sb)):
                                nc.tensor.matmul(
                                    pair[:, h, :], k_sb[:, jsl], qp[:, qss],
                                    start=True, stop=True,
                                )
                            import os as _os3
                            if _os3.environ.get("CONSTET") == "1":
                                ets[jt] = cet
                            else:
                                et = exp_pool.tile([P, HPC, QW], bf16)
                                if _os3.environ.get("EXPSPLIT") == "1":
                                    for h in range(HPC):
                                        nc.scalar.activation(
                                            et[:, h, :], pair[:, h, :],
                                            mybir.ActivationFunctionType.Exp, scale=0.125,
                                        )
                                else:
                                    nc.scalar.activation(
                                        et[:], pair[:],
                                        mybir.ActivationFunctionType.Exp, scale=0.125,
                                    )
                                ets[jt] = et
                        ja = jt - SKEW
                        if ja >= 0:
                            et = ets.pop(ja)
                            for h in range(HPC):
                                nc.tensor.matmul(
                                    pav[h][:],
                                    v_sb[:, b * SJT + ja, h * (DK + 1) : (h + 1) * (DK + 1)],
                                    et[:, h, :],
                                    start=(ja == 0), stop=(ja == SJT - 1),
                                )
                        drain(1)
                        if jt == 9:
                            nb, nqc = (b, qc + 1) if qc + 1 < NQC2 else (b + 1, 0)
                            if nb < B:
                                force(("qk_q", nb, nqc))
                    for h in range(HPC):
                        if NOCHAIN:
                            continue
                        import os as _os
                        if _os.environ.get("LNCHAIN", "1") == "1":
                            # reciprocal of the softmax sums via exp(-ln(s)) on
                            # ScalarE (same activation-table set as the score
                            # exp), keeping the slow DVE InstReciprocal off the
                            # critical path entirely.
                            lnr = rc_pool.tile([1, QW], f32)
                            nc.scalar.activation(
                                lnr[:], pav[h][DK : DK + 1, :],
                                mybir.ActivationFunctionType.Ln)
                            rec = rc_pool.tile([1, QW], f32, name="rec")
                            nc.scalar.activation(
                                rec[:], lnr[:],
                                mybir.ActivationFunctionType.Exp, scale=-1.0)
                            bc = bc_pool.tile([DK, QW], f32)
                            nc.gpsimd.partition_broadcast(bc[:], rec[:])
                            nc.vector.tensor_mul(
                                attn_c[h * DK : (h + 1) * DK, :], pav[h][0:DK, :], bc[:])
                            continue
                        if _os.environ.get("DIVCHAIN", "0") == "1":
                            # evict pav to SBUF first: frees the PSUM slot after
                            # one op; the normalization chain then runs on SBUF
                            # with no PSUM slot held.
                            un = un_pool.tile([DK + 1, QW], f32)
                            nc.scalar.copy(un[:], pav[h][:])
                            rc = rc_pool.tile([1, QW], f32)
                            nc.vector.reciprocal(rc[:], un[DK : DK + 1, :])
                            bc = bc_pool.tile([DK, QW], f32)
                            nc.gpsimd.partition_broadcast(bc[:], rc[:])
                            nc.vector.tensor_mul(
                                attn_c[h * DK : (h + 1) * DK, :], un[0:DK, :], bc[:])
                            continue
                        rc = rc_pool.tile([1, QW], f32)
                        nc.vector.reciprocal(rc[:], pav[h][DK : DK + 1, :])
                        bc = bc_pool.tile([DK, QW], f32)
                        if _os.environ.get("NOBCAST") == "1":
                            nc.vector.memset(bc[:], 1.0)   # timing-only variant
                        else:
                            nc.gpsimd.partition_broadcast(bc[:], rc[:])
                        nc.vector.tensor_mul(
                            attn_c[h * DK : (h + 1) * DK, :], pav[h][0:DK, :], bc[:])
                    if not NOWO:
                        for i2 in range(QW // P):
                            pending.append(("wo", attn_c, b, qc, i2))

            drain(len(pending))


def _build(loop=1):
    nc = bacc.Bacc("TRN2", target_bir_lowering=False, debug=False,
                   num_devices=N_CORES)
    xq = nc.dram_tensor("xq_t", [D_MODEL, BS], bf16, kind="ExternalInput").ap()
    xk = nc.dram_tensor("xk_t", [D_MODEL, BS], bf16, kind="ExternalInput").ap()
    xv = nc.dram_tensor("xv_t", [D_MODEL, BS], bf16, kind="ExternalInput").ap()
    wq = nc.dram_tensor("wq_t", [P, D_MODEL], bf16, kind="ExternalInput").ap()
    wk = nc.dram_tensor("wk_t", [P, D_MODEL], bf16, kind="ExternalInput").ap()
    wv = nc.dram_tensor("wv_t", [P, D_MODEL], bf16, kind="ExternalInput").ap()
    wo = nc.dram_tensor("wo_t", [DPC, D_MODEL], bf16, kind="ExternalInput").ap()
    bq = nc.dram_tensor("bq", [DPC, 1], f32, kind="ExternalInput").ap()
    bk = nc.dram_tensor("bk", [DPC, 1], f32, kind="ExternalInput").ap()
    out = nc.dram_tensor("out_p", [BS, D_MODEL], f32, kind="ExternalOutput").ap()

    with tile.TileContext(nc) as tc:
        _emit(tc, (xq, xk, xv, wq, wk, wv, wo, bq, bk, out), loop=loop)
    nc.compile()
    return nc


_cache = {}


def _get_nc(loop=1):
    import os
    key = (loop, os.environ.get("SKEW", "2"), os.environ.get("PPPAIR", "2"),
           os.environ.get("PPAV", "3"), os.environ.get("PPBLK", "1"), os.environ.get("NOEXP", ""), os.environ.get("EXPSPLIT", ""), os.environ.get("CONSTET", ""), os.environ.get("PROJONLY", ""), os.environ.get("NOX", ""), os.environ.get("NOPROJ", ""), os.environ.get("NOWO", ""), os.environ.get("NOCHAIN", ""), os.environ.get("DIVCHAIN", "0"), os.environ.get("LNCHAIN", "1"), os.environ.get("WOACT", "0"), os.environ.get("LNCHAIN", "1"))
    if key not in _cache:
        _cache[key] = _build(loop)
    return _cache[key]


def _wprep(w_slice):
    # [dpc, D] weight slice -> (ctr%128, ctr//128, dpc) contiguous per
    # partition so the device load uses 2KB descriptors instead of 256B.
    wt = np.ascontiguousarray(np.asarray(w_slice).T)   # [D, dpc]
    wt = wt.reshape(NF, P, DPC).transpose(1, 0, 2)     # [128, 8, 128]
    return np.ascontiguousarray(wt.reshape(P, D_MODEL)).astype(BF16)


def _make_in_maps(Q, K, V, Wq, bq, Wk, bk, Wv, bv, Wo, bo):
    xq_t = np.ascontiguousarray(np.asarray(Q, np.float32).reshape(BS, D_MODEL).T).astype(BF16)
    xk_t = np.ascontiguousarray(np.asarray(K, np.float32).reshape(BS, D_MODEL).T).astype(BF16)
    xv_t = np.ascontiguousarray(np.asarray(V, np.float32).reshape(BS, D_MODEL).T).astype(BF16)
    in_maps = []
    for c in range(N_CORES):
        sl = slice(c * DPC, (c + 1) * DPC)
        in_maps.append({
            "xq_t": xq_t, "xk_t": xk_t, "xv_t": xv_t,
            "wq_t": _wprep(np.asarray(Wq)[sl]),
            "wk_t": _wprep(np.asarray(Wk)[sl]),
            "wv_t": _wprep(np.asarray(Wv)[sl]),
            "wo_t": np.ascontiguousarray(np.asarray(Wo)[:, sl].T).astype(BF16),
            "bq": np.asarray(bq, np.float32)[sl].reshape(DPC, 1).copy(),
            "bk": np.asarray(bk, np.float32)[sl].reshape(DPC, 1).copy(),
        })
    return in_maps


def kernel(Q, K, V, Wq, bq, Wk, bk, Wv, bv, Wo, bo):
    nc = _get_nc()
    in_maps = _make_in_maps(Q, K, V, Wq, bq, Wk, bk, Wv, bv, Wo, bo)
    res = bass_utils.run_bass_kernel_spmd(nc, in_maps, core_ids=list(range(N_CORES)))
    acc = np.zeros((BS, D_MODEL), np.float32)
    for c in range(N_CORES):
        acc += res.results[c]["out_p"]
    corr = (np.asarray(bv, np.float64) @ np.asarray(Wo, np.float64).T
            + np.asarray(bo, np.float64)).astype(np.float32)
    return (acc + corr[None, :]).reshape(B, S, D_MODEL).astype(np.float32)

